# revision 1
# baseline (speedup 1.0000x reference)
"""Bass/Trainium2 kernel for full attention: softmax(Q K^T / d_k) V.

Shapes (hardcoded): Q [8192, 128], K [8192, 128], V [8192, 128] -> out [8192, 128].
Sharding: Q rows split across 8 NeuronCores (1024 queries/core); K, V replicated.

Per-core algorithm (transposed orientation; F-form softmax for fp8 DoubleRow):
  softmax(S)V = (sum_m E V) / (sum_m E),  E = exp(s/128)
              = (colsum(V) + sum_m F V) / (M + sum_m F),  F = E - 1
  F is small (|F| <= ~0.5), so fp8e4m3 quantizes it ~10x better than E.
  - S^T chunk MMs in bf16 (Q^T, K^T cast on host):  [128m, 512n] f32 PSUM
  - ScalarE: t = exp(S^T/128) -> bf16
  - DVE:     F = t - 1        -> fp8e4m3
  - PE DoubleRow MMs (fp8, 256-key contraction per MM, 0.5 cyc/row):
      o   += (V8 pair).T @ F pair   and  o += (dV8 pair).T @ F pair
      sums += ones.T @ F pair
    where V8 = fp8(V), dV8 = fp8(V - V8) (residual recovers ~bf16 V accuracy)
  - colsum(V) added to o via a tiny f32 outer-product MM; denominator gets
    +8192 for free in the ScalarE PSUM->SBUF copy (Copy with bias).
  - normalize: 1/d is affine in s near d=M (one Newton step from 1/M).
Host: gather + transpose per-core O^T -> full [8192, 128].
"""

import numpy as np
import ml_dtypes

import concourse.bass as bass
import concourse.mybir as mybir
import concourse.tile as tile
from concourse.bass_utils import run_bass_kernel_spmd

N, M, D = 8192, 8192, 128
NCORES = 8
NLOC = N // NCORES            # 1024 queries per core
NT = 512                      # query tile (f32 PSUM bank limit)
MCHUNK = 128                  # key chunk (partition dim of S^T tiles)
NMC = M // MCHUNK             # 64 chunks
NPAIR = NMC // 2              # 32 chunk pairs (DoubleRow contraction = 256)
SCALE = 1.0 / D
STRIPE = 1024                 # prelude DMA stripe width (keys)

F32 = mybir.dt.float32
BF16 = mybir.dt.bfloat16
FP8 = mybir.dt.float8e4
EXP = mybir.ActivationFunctionType.Exp
COPY = mybir.ActivationFunctionType.Copy
DR = mybir.MatmulPerfMode.DoubleRow
SUB = mybir.AluOpType.subtract

# chunks whose exp runs as a DVE quadratic instead of ScalarE (engine balance)
DVE_CHUNKS = frozenset()

TRACE = False                 # test.py sets True to capture NTFF profile
LAST_RESULT = {}              # test.py reads exec_time_ns etc.


def build():
    nc = bass.Bass()
    QT_d = nc.dram_tensor("QT", [D, NLOC], BF16, kind="ExternalInput")
    KT_d = nc.dram_tensor("KT", [D, M], BF16, kind="ExternalInput")
    VS_d = nc.dram_tensor("VS", [D, M], FP8, kind="ExternalInput")
    DVS_d = nc.dram_tensor("DVS", [D, M], FP8, kind="ExternalInput")
    CSHL_d = nc.dram_tensor("CSHL", [2, D], BF16, kind="ExternalInput")
    OT_d = nc.dram_tensor("OT", [D, NLOC], F32, kind="ExternalOutput")

    with tile.TileContext(nc) as tc:
        with (
            tc.tile_pool(name="sb", bufs=1) as sb,
            tc.tile_pool(name="ps", bufs=2, space="PSUM") as ps,
            tc.tile_pool(name="po", bufs=1, space="PSUM") as po,
            tc.tile_pool(name="psm", bufs=1, space="PSUM") as psm,
        ):
            const = big = tp = fpool = outp = sb
            # warm the exp table during the prelude DMAs
            warm = const.tile([1, 64], F32)
            nc.vector.memset(warm[:], 0.0)
            warm_o = const.tile([1, 64], BF16)
            nc.scalar.activation(warm_o[:], warm[:], EXP, scale=SCALE)

            # DR weight APs need group stride %16 == 0: ones at cols 0,16
            ones8 = const.tile([128, 32], FP8)
            nc.vector.memset(ones8[:], 1.0)
            ones_row = const.tile([1, NT], BF16)
            nc.vector.memset(ones_row[:], 1.0)
            ones_col = const.tile([1, 128], BF16)
            nc.vector.memset(ones_col[:], 1.0)


            KT = big.tile([128, M], BF16)
            QT = big.tile([128, NLOC], BF16)
            VS = big.tile([128, M], FP8)
            DVS = big.tile([128, M], FP8)
            CSH = big.tile([1, 128], BF16)
            CSL = big.tile([1, 128], BF16)

            nc.sync.dma_start(QT[:], QT_d[:])
            nc.gpsimd.dma_start(CSH[:], CSHL_d[0:1, :])
            nc.gpsimd.dma_start(CSL[:], CSHL_d[1:2, :])
            # small first K stripe so the first S-MM starts ASAP
            kt_bounds = [0, 256, 1024, 2048, 4096, 6144, 8192]
            for a, b in zip(kt_bounds[:-1], kt_bounds[1:]):
                nc.sync.dma_start(KT[:, a:b], KT_d[:, a:b])
            for s in range(4):
                sl = slice(s * 2048, (s + 1) * 2048)
                nc.gpsimd.dma_start(VS[:, sl], VS_d[:, sl])
                nc.gpsimd.dma_start(DVS[:, sl], DVS_d[:, sl])

            o_ps = po.tile([128, NLOC], F32, tag="po")       # 2 banks (nt halves)
            s_ps = psm.tile([1, NLOC], F32, tag="psm")       # 2 banks


            def dr_mms(g):
                """DoubleRow MMs for chunk pair g (consumes F pair tile)."""
                fpair = fpairs[g]
                rhs_all = fpair[:].rearrange("p (i n) -> p i n", i=2)
                vsl = slice(g * 256, (g + 1) * 256)
                v_lhs = VS[:, vsl].rearrange("p (i v) -> p i v", i=2)
                dv_lhs = DVS[:, vsl].rearrange("p (i v) -> p i v", i=2)
                one_lhs = ones8[:].rearrange("p (i v) -> p i v", i=2)[:, :, 0:1]
                for nt in range(2):
                    rhs = rhs_all[:, :, nt * NT : (nt + 1) * NT]
                    nc.tensor.matmul(
                        s_ps[:, nt * NT : (nt + 1) * NT], one_lhs, rhs,
                        start=(g == 0), stop=False,
                        perf_mode=DR, skip_group_check=True,
                    )
                for nt in range(2):
                    rhs = rhs_all[:, :, nt * NT : (nt + 1) * NT]
                    osl = o_ps[:, nt * NT : (nt + 1) * NT]
                    nc.tensor.matmul(
                        osl, v_lhs, rhs, start=(g == 0), stop=False,
                        perf_mode=DR, skip_group_check=True,
                    )
                    nc.tensor.matmul(
                        osl, dv_lhs, rhs, start=False, stop=False,
                        perf_mode=DR, skip_group_check=True,
                    )

            fpairs = {}
            for c in range(NMC):
                g, j = divmod(c, 2)
                sp = ps.tile([128, NLOC], F32, tag="sp")
                for nt in range(2):
                    nc.tensor.matmul(
                        sp[:, nt * NT : (nt + 1) * NT],
                        KT[:, c * 128 : (c + 1) * 128],
                        QT[:, nt * NT : (nt + 1) * NT],
                        start=True,
                        stop=True,
                    )
                if j == 0:
                    fpairs[g] = fpool.tile(
                        [128, 2 * NLOC], FP8, tag="f", name=f"fpair{g}", bufs=10
                    )
                fsl = fpairs[g][:, j * NLOC : (j + 1) * NLOC]
                if c in DVE_CHUNKS:
                    # DVE quadratic: F2 = x*(1 + x/2), x = s*SCALE (err x^3/6)
                    # pass 1 releases the PSUM tile immediately
                    xb = tp.tile([128, NLOC], BF16, tag="xb", bufs=3)
                    nc.vector.tensor_scalar(
                        xb[:], sp[:], SCALE, None, mybir.AluOpType.mult
                    )
                    w = tp.tile([128, NLOC], BF16, tag="w", bufs=3)
                    nc.vector.tensor_scalar(
                        w[:], xb[:], 0.5, 1.0,
                        mybir.AluOpType.mult, mybir.AluOpType.add,
                    )
                    nc.vector.tensor_mul(fsl, xb[:], w[:])
                else:
                    t = tp.tile([128, NLOC], BF16, tag="t", bufs=6)
                    nc.scalar.activation(t[:], sp[:], EXP, scale=SCALE)
                    nc.vector.tensor_scalar(fsl, t[:], 1.0, None, SUB)
                # lag DR consumption ~2 pairs, flush two pairs per batch to
                # halve PE array S<->DR turnarounds
                if c % 4 == 3 and c >= 7:
                    dr_mms((c - 7) // 2)
                    dr_mms((c - 7) // 2 + 1)
                if c == 7:
                    # numerator += colsum(V) as two bf16 outer products
                    # (cs = csh + csl, exact to ~2^-16)
                    for nt in range(2):
                        osl = o_ps[:, nt * NT : (nt + 1) * NT]
                        nc.tensor.matmul(osl, CSH[:], ones_row[:],
                                         start=False, stop=False,
                                         skip_group_check=True)
                        nc.tensor.matmul(osl, CSL[:], ones_row[:],
                                         start=False, stop=False,
                                         skip_group_check=True)
            for g in (NPAIR - 2, NPAIR - 1):
                fpair = fpairs[g]
                rhs_all = fpair[:].rearrange("p (i n) -> p i n", i=2)
                one_lhs = ones8[:].rearrange("p (i v) -> p i v", i=2)[:, :, 0:1]
                for nt in range(2):
                    nc.tensor.matmul(
                        s_ps[:, nt * NT : (nt + 1) * NT], one_lhs,
                        rhs_all[:, :, nt * NT : (nt + 1) * NT],
                        start=False, stop=(g == NPAIR - 1),
                        perf_mode=DR, skip_group_check=True,
                    )
            for g in (NPAIR - 2, NPAIR - 1):
                fpair = fpairs[g]
                rhs_all = fpair[:].rearrange("p (i n) -> p i n", i=2)
                vsl = slice(g * 256, (g + 1) * 256)
                v_lhs = VS[:, vsl].rearrange("p (i v) -> p i v", i=2)
                dv_lhs = DVS[:, vsl].rearrange("p (i v) -> p i v", i=2)
                for nt in range(2):
                    rhs = rhs_all[:, :, nt * NT : (nt + 1) * NT]
                    osl = o_ps[:, nt * NT : (nt + 1) * NT]
                    nc.tensor.matmul(osl, v_lhs, rhs, start=False, stop=False,
                                     perf_mode=DR, skip_group_check=True)
                    nc.tensor.matmul(osl, dv_lhs, rhs, start=False,
                                     stop=(g == NPAIR - 1 and nt == 1),
                                     perf_mode=DR, skip_group_check=True)

            # normalize: o / (M + s), s = sum F. d = M + s is within ~1% of M,
            # so one fixed-seed Newton step r = r0(2 - d*r0), r0 = 1/M, gives
            # 1/d to ~7e-5 rel -- and it is AFFINE in s: r = 1/M - s/M^2.
            for nt in range(2):
                sl = slice(nt * NT, (nt + 1) * NT)
                s_sb = outp.tile([1, NT], BF16, tag="ssb", bufs=2)
                nc.scalar.activation(s_sb[:], s_ps[:, sl], COPY)
                bc_ps = ps.tile([128, NLOC], F32, tag="sp")
                nc.tensor.matmul(
                    bc_ps[:, 0:NT], ones_col[:], s_sb[:], start=True, stop=True
                )
                rec_bc = outp.tile([128, NT], F32, tag="bc", bufs=2)
                nc.vector.tensor_scalar(
                    rec_bc[:], bc_ps[:, 0:NT],
                    -1.0 / (float(M) * float(M)), 1.0 / float(M),
                    mybir.AluOpType.mult, mybir.AluOpType.add,
                )
                o_sb = outp.tile([128, NT], F32, tag="osb", bufs=2)
                nc.vector.tensor_mul(o_sb[:], o_ps[:, sl], rec_bc[:])
                nc.sync.dma_start(OT_d[:, sl], o_sb[:])

    return nc


def _fix_multiwaits(nc):
    """Walrus encodes at most one sem-wait on Matmult/Activation/DMACopy
    structs. Tile emits redundant same-engine waits (engines complete
    in order; the HW DRAIN covers intra-engine output hazards) - drop
    them so every such instruction carries a single wait."""
    eng_sem = {
        "EngineType.Activation": "Activation",
        "EngineType.PE": "PE",
        "EngineType.DVE": "DVE",
        "EngineType.Pool": "Pool",
        "EngineType.SP": "SP",
    }
    fn = nc.m.functions[0]
    leftover = []
    for blk in fn.blocks:
        for i in blk.instructions:
            si = getattr(i, "sync_info", None)
            if not si or not si.on_wait or len(si.on_wait) < 2:
                continue
            own = eng_sem.get(str(getattr(i, "engine", "")), "???")
            keep = [w for w in si.on_wait if not w.ant_name.startswith(own + "_")]
            if len(keep) < len(si.on_wait) and len(keep) <= 1:
                si.on_wait = keep
            elif len(si.on_wait) > 1:
                leftover.append((blk, i))
    # move extra waits onto standalone same-engine NoOps inserted before
    for blk, i in leftover:
        si = i.sync_info
        extra, keep = list(si.on_wait[:-1]), [si.on_wait[-1]]
        idx = next(k for k, x in enumerate(blk.instructions) if x.name == i.name)
        nops = []
        for w_i, w in enumerate(extra):
            nop = mybir.InstNoOp(name=f"W-{i.name}-{w_i}", ins=[], outs=[])
            nop.engine = i.engine
            nsi = mybir.SyncInfo(on_wait=[w], on_update=[])
            nop.sync_info = nsi
            nops.append(nop)
        blk.instructions[idx:idx] = nops
        si.on_wait = keep


_NC = None
_HOST_PREP = None


def _prep_host(K, V):
    """Cast/layout K and V once: KT bf16, V fp8 stripes + fp8 residual."""
    KT = np.ascontiguousarray(K.T).astype(ml_dtypes.bfloat16)
    V8 = V.astype(ml_dtypes.float8_e4m3)
    dV = (V - V8.astype(np.float32)).astype(ml_dtypes.float8_e4m3)
    # VS[p, c*128+v] = V[c*128+p, v]
    VS = np.ascontiguousarray(
        V8.reshape(NMC, 128, 128).transpose(1, 0, 2).reshape(128, M)
    )
    DVS = np.ascontiguousarray(
        dV.reshape(NMC, 128, 128).transpose(1, 0, 2).reshape(128, M)
    )
    CS = V.sum(axis=0, dtype=np.float64).astype(np.float32)
    CSH = CS.astype(ml_dtypes.bfloat16)
    CSL = (CS - CSH.astype(np.float32)).astype(ml_dtypes.bfloat16)
    CSHL = np.ascontiguousarray(np.stack([CSH, CSL], axis=0))
    return KT, VS, DVS, CSHL


def kernel(Q, K, V):
    global _NC, _HOST_PREP, LAST_RESULT
    Q = np.asarray(Q, dtype=np.float32)
    K = np.asarray(K, dtype=np.float32)
    V = np.asarray(V, dtype=np.float32)
    if _NC is None:
        _NC = build()
        _fix_multiwaits(_NC)
    KT, VS, DVS, CSHL = _prep_host(K, V)
    QTb = np.ascontiguousarray(Q.T).astype(ml_dtypes.bfloat16)
    in_maps = [
        {
            "QT": np.ascontiguousarray(QTb[:, c * NLOC : (c + 1) * NLOC]),
            "KT": KT,
            "VS": VS,
            "DVS": DVS,
            "CSHL": CSHL,
        }
        for c in range(NCORES)
    ]
    if TRACE:
        _install_ntff_hook()
    res = run_bass_kernel_spmd(
        _NC, in_maps, core_ids=list(range(NCORES)), trace=TRACE
    )
    LAST_RESULT = {
        "exec_time_ns": res.exec_time_ns,
        "mean_exec_time_ns": res.mean_exec_time_ns,
        "trace": res.instructions_and_trace,
        "profile_json": res.profile_json,
    }
    out = np.concatenate([r["OT"].T for r in res.results], axis=0)
    return np.ascontiguousarray(out.astype(np.float32))


def _install_ntff_hook():
    """Shim the missing antenv.axon_hooks module so run_bass_kernel_spmd's
    trace path can drive NTFF capture through libaxon_pjrt.so directly."""
    import sys
    import types

    try:
        from antenv.axon_hooks import get_axon_ntff_profile_hook  # noqa: F401
        return
    except ImportError:
        pass
    sys.path.insert(0, "/root/.axon_site")
    from trn_agent_boot.trn_boot import _ntff_profile_via_ctypes

    hook = _ntff_profile_via_ctypes("/opt/axon/libaxon_pjrt.so")
    mod = types.ModuleType("antenv.axon_hooks")
    mod.get_axon_ntff_profile_hook = lambda: hook
    mod.set_axon_ntff_profile_hook = lambda h: None
    sys.modules["antenv.axon_hooks"] = mod

